# revision 1
# baseline (speedup 1.0000x reference)
"""VQ codebook cosine-similarity softmax kernel for Trainium2 (8 NeuronCores).

Computes softmax(cos_sim(batch, centroids)) for batch [131072, 1024] f32 and
centroids [256, 1024] f32, data-parallel over the batch dim across 8 cores.

Per-core pipeline (16384 rows):
  - SWDGE cast-DMA loads x tiles HBM f32 -> SBUF fp16 (halves SBUF traffic,
    enables full-rate fp16 matmuls; fp32 PSUM accumulation keeps rel err ~5e-5)
  - PE transposes each [128,128] fp16 block (x needs D on partitions for the
    matmul contraction); PSUM->SBUF copyback split between DVE and ACT
  - PE matmul: weights = xT block [128d,128n], moving = cnT [128d,256k],
    accumulating over 8 d-chunks into PSUM f32 [128n, 256k]
  - row norms on DVE: tensor_tensor_reduce(x*x) then rsqrt via the
    0x5f3759df bit trick + 3 Newton steps (keeps Ln/Sqrt off ACT so only
    the Exp table set ever loads)
  - softmax: logits = cos in [-1,1] so no max-subtraction needed;
    ACT Exp(scale=1/||x||) with accum_out giving the denominator,
    DVE reciprocal + ACT Copy(scale=1/denom) for the final normalize
"""

import os
import sys

if "/opt/trn_rl_repo" not in sys.path:
    sys.path.insert(0, "/opt/trn_rl_repo")

import numpy as np

RSQRT_MODE = os.environ.get("KM_RSQRT_MODE", "bit")  # bit | act
MUL_MODE = os.environ.get("KM_MUL_MODE", "act")  # act | dve
COPY_SPLIT = os.environ.get("KM_COPY_SPLIT", "0") == "1"  # DVE+ACT vs DVE only
SQ_MODE = os.environ.get("KM_SQ_MODE", "sts")  # sts | ttr | act
# NOTE: ttr (tensor_tensor_reduce) compiles and simulates fine but faults the
# device at runtime — do not use. sts (scalar_tensor_tensor + accum) works.
# SQ_SPLIT: columns of each row handled by DVE (sts); the rest go to ACT
# (Square). Both run at 1 elem/cycle/lane, so this splits the norm pass
# across the two engines. 0 = all ACT, 1024 = all DVE.
SQ_SPLIT = int(os.environ.get("KM_SQ_SPLIT", "640"))
EARLY_CLOSE = os.environ.get("KM_EARLY_CLOSE", "1") == "1"
SPS_BUFS = int(os.environ.get("KM_SPS_BUFS", "4"))
XT_BUFS = int(os.environ.get("KM_XT_BUFS", "4"))
E_BUFS = int(os.environ.get("KM_E_BUFS", "6"))
DEN_BUFS = int(os.environ.get("KM_DEN_BUFS", "6"))
NRM_BUFS = int(os.environ.get("KM_NRM_BUFS", "4"))


N, D, K = 131072, 1024, 256
NCORES = 8
NPC = N // NCORES  # rows per core
P = 128  # partitions / tile rows
XB = 4  # row-tiles per load/store DMA batch
G = 16  # row-tiles per norm group (batched rsqrt)
F1 = 832  # copyback columns done by DVE (rest by ACT)

RSQRT_MAGIC = 0x5F3759DF


def build_bass(npc=NPC):
    """Build the single-core SPMD program; every core runs this with its own
    x shard. Returns the compiled Bacc object."""
    from contextlib import ExitStack

    import concourse.bacc as bacc
    import concourse.mybir as mybir
    import concourse.tile as tile
    from concourse.masks import make_identity

    dt = mybir.dt
    AFT = mybir.ActivationFunctionType
    Alu = mybir.AluOpType

    nt = npc // P  # row tiles
    assert npc % (P * XB) == 0
    ngroups = (nt + G - 1) // G

    nc = bacc.Bacc(
        "TRN2", target_bir_lowering=False, debug=False, num_devices=NCORES
    )
    x_d = nc.dram_tensor("x", [npc, D], dt.float32, kind="ExternalInput")
    c_d = nc.dram_tensor("c", [K, D], dt.float32, kind="ExternalInput")
    o_d = nc.dram_tensor("o", [npc, K], dt.float32, kind="ExternalOutput")

    ND = D // P  # d-chunks (8)

    def emit_rsqrt(nc, dst, src, scratch_a, scratch_b, w):
        """dst[:, :w] = 1/sqrt(src[:, :w]).

        bit mode: 0x5f3759df bit trick + 3 Newton steps, all on DVE.
        act mode: exp(-0.5*ln(src)) seed on ACT + 1 Newton step on DVE.
        """
        if RSQRT_MODE == "bit":
            srci = src.bitcast(dt.int32)
            dsti = dst.bitcast(dt.int32)
            nc.vector.tensor_scalar(
                dsti, srci, 1, None, Alu.logical_shift_right
            )
            # magic - x == (x ^ 0xffffffff) + (magic + 1)  (avoids int negate)
            nc.vector.tensor_scalar(dsti, dsti, -1, None, Alu.bitwise_xor)
            nc.vector.tensor_scalar(dsti, dsti, RSQRT_MAGIC + 1, None, Alu.add)
            niter = 3
        else:
            nc.scalar.activation(scratch_a, src, AFT.Ln)
            nc.scalar.activation(dst, scratch_a, AFT.Exp, scale=-0.5)
            niter = 1
        for _ in range(niter):
            nc.vector.tensor_tensor(scratch_a, dst, dst, Alu.mult)
            nc.vector.tensor_tensor(scratch_b, scratch_a, src, Alu.mult)
            nc.vector.tensor_scalar(
                scratch_b, scratch_b, -0.5, 1.5, Alu.mult, Alu.add
            )
            nc.vector.tensor_tensor(dst, dst, scratch_b, Alu.mult)

    with tile.TileContext(nc) as tc, ExitStack() as ctx:
        const = ctx.enter_context(tc.tile_pool(name="const", bufs=1))
        ident = const.tile([P, P], dt.float16)
        make_identity(nc, ident[:])

        # cnT: [128 (d within chunk), ND * K] fp16; chunk b at cols [K*b, K*b+K)
        cnT = const.tile([P, ND * K], dt.float16)
        # per-tile squared row norms (partial sums: a=DVE part, b=ACT part)
        n2a = const.tile([P, max(nt, 1)], dt.float32)
        n2b = const.tile([P, max(nt, 1)], dt.float32)
        # per-tile softmax denominators and their reciprocals
        denscols = const.tile([P, max(nt, 1)], dt.float32)
        rdenscols = const.tile([P, max(nt, 1)], dt.float32)

        # ---- centroid prep (one-time, ~1MB); pools close before main loop ----
        with ExitStack() as _cstack:
            cctx = _cstack if EARLY_CLOSE else ctx
            cprep = cctx.enter_context(tc.tile_pool(name="cprep", bufs=2))
            cpsum = cctx.enter_context(
                tc.tile_pool(name="cpsum", bufs=2, space="PSUM")
            )
            for h in range(K // P):  # 2 halves of the K=256 centroids
                c32 = cprep.tile([P, D], dt.float32, tag="c32")
                nc.sync.dma_start(c32[:], c_d.ap()[P * h : P * (h + 1), :])
                csq = cprep.tile([P, D], dt.float32, tag="csq")
                cn2 = cprep.tile([P, 1], dt.float32, tag="cn2")
                if SQ_MODE == "ttr":
                    nc.vector.tensor_tensor_reduce(
                        csq[:], c32[:], c32[:], 1.0, 0.0, Alu.mult, Alu.add,
                        accum_out=cn2[:],
                    )
                elif SQ_MODE == "sts":
                    nc.vector.scalar_tensor_tensor(
                        csq[:], c32[:], 1.0, c32[:], Alu.mult, Alu.mult,
                        accum_out=cn2[:],
                    )
                else:
                    nc.scalar.activation(
                        csq[:], c32[:], AFT.Square, accum_out=cn2[:]
                    )
                crn = cprep.tile([P, 1], dt.float32, tag="crn")
                csa = cprep.tile([P, 1], dt.float32, tag="csa")
                csb = cprep.tile([P, 1], dt.float32, tag="csb")
                emit_rsqrt(nc, crn[:], cn2[:], csa[:], csb[:], 1)
                cn16 = cprep.tile([P, D], dt.float16, tag="cn16")
                nc.vector.tensor_scalar_mul(cn16[:], c32[:], crn[:])
                for b in range(ND):
                    pt = cpsum.tile([P, P], dt.float16, tag="ct_ps")
                    nc.tensor.transpose(
                        pt[:], cn16[:, P * b : P * (b + 1)], ident[:]
                    )
                    nc.vector.tensor_copy(
                        cnT[:, K * b + P * h : K * b + P * h + P], pt[:]
                    )

        # ---- main loop ----
        x16_pool = ctx.enter_context(tc.tile_pool(name="x16", bufs=2 * G // XB))
        xt_pool = ctx.enter_context(tc.tile_pool(name="xt", bufs=XT_BUFS))
        sq_pool = ctx.enter_context(tc.tile_pool(name="sq", bufs=2))
        e_pool = ctx.enter_context(tc.tile_pool(name="e", bufs=E_BUFS))
        pm_pool = ctx.enter_context(tc.tile_pool(name="pm", bufs=3))
        nrm_pool = ctx.enter_context(tc.tile_pool(name="nrm", bufs=NRM_BUFS))
        den_pool = ctx.enter_context(tc.tile_pool(name="den", bufs=DEN_BUFS))
        tps_pool = ctx.enter_context(
            tc.tile_pool(name="tps", bufs=2, space="PSUM")
        )
        sps_pool = ctx.enter_context(
            tc.tile_pool(name="sps", bufs=SPS_BUFS, space="PSUM")
        )

        for g in range(ngroups):
            t0 = g * G
            t1 = min(t0 + G, nt)
            gtiles = range(t0, t1)
            gw = t1 - t0
            # 1) cast-loads (XB row-tiles per DMA)
            xmacs = {}
            for tm in range(t0 // XB, (t1 + XB - 1) // XB):
                xm = x16_pool.tile([P, XB * D], dt.float16, tag="xm")
                src = x_d.ap()[P * XB * tm : P * XB * (tm + 1), :].rearrange(
                    "(s p) d -> p s d", s=XB
                )
                nc.gpsimd.dma_start(
                    xm[:].rearrange("p (s d) -> p s d", s=XB), src
                )
                xmacs[tm] = xm
            # 2) row norms^2, split column-wise across DVE (sts) and ACT (Square)
            sd = max(0, min(D, SQ_SPLIT))
            for t in gtiles:
                xm = xmacs[t // XB]
                xs = xm[:, D * (t % XB) : D * (t % XB + 1)]
                if sd > 0:
                    sqa = sq_pool.tile([P, D], dt.float16, tag="sqa")
                    nc.vector.scalar_tensor_tensor(
                        sqa[:, :sd], xs[:, :sd], 1.0, xs[:, :sd],
                        Alu.mult, Alu.mult, accum_out=n2a[:, t : t + 1],
                    )
                if sd < D:
                    sqb = sq_pool.tile([P, D], dt.float16, tag="sqb")
                    nc.scalar.activation(
                        sqb[:, sd:], xs[:, sd:], AFT.Square,
                        accum_out=n2b[:, t : t + 1],
                    )
            # 3) batched rsqrt for the group's norms
            rng = nrm_pool.tile([P, G], dt.float32, tag="rng")
            nsa = nrm_pool.tile([P, G], dt.float32, tag="nsa")
            nsb = nrm_pool.tile([P, G], dt.float32, tag="nsb")
            n2s = nrm_pool.tile([P, G], dt.float32, tag="n2s")
            if sd == 0:
                n2src = n2b[:, t0:t1]
            elif sd == D:
                n2src = n2a[:, t0:t1]
            else:
                nc.vector.tensor_tensor(
                    n2s[:, :gw], n2a[:, t0:t1], n2b[:, t0:t1], Alu.add
                )
                n2src = n2s[:, :gw]
            emit_rsqrt(nc, rng[:, :gw], n2src, nsa[:, :gw], nsb[:, :gw], gw)
            # 4) per XB-block: transpose -> matmul -> exp, then batched
            #    reciprocal of the denominators, normalize, store
            for tm in range(t0 // XB, (t1 + XB - 1) // XB):
                bt0 = max(t0, tm * XB)
                bt1 = min(t1, (tm + 1) * XB)
                for t in range(bt0, bt1):
                    xm = xmacs[t // XB]
                    xs = xm[:, D * (t % XB) : D * (t % XB + 1)]
                    tps = tps_pool.tile([P, D], dt.float16, tag="tps")
                    for b in range(ND):
                        nc.tensor.transpose(
                            tps[:, P * b : P * (b + 1)],
                            xs[:, P * b : P * (b + 1)],
                            ident[:],
                        )
                    xt = xt_pool.tile([P, D], dt.float16, tag="xt")
                    if COPY_SPLIT:
                        nc.vector.tensor_copy(xt[:, :F1], tps[:, :F1])
                        nc.scalar.copy(xt[:, F1:], tps[:, F1:])
                    else:
                        nc.vector.tensor_copy(xt[:], tps[:])
                    sps = sps_pool.tile([P, K], dt.float32, tag="sps")
                    for b in range(ND):
                        nc.tensor.matmul(
                            sps[:],
                            xt[:, P * b : P * (b + 1)],
                            cnT[:, K * b : K * (b + 1)],
                            start=(b == 0),
                            stop=(b == ND - 1),
                        )
                    e = e_pool.tile([P, K], dt.float32, tag="e")
                    den = den_pool.tile([P, 1], dt.float32, tag="den")
                    j = t - t0
                    nc.scalar.activation(
                        e[:], sps[:], AFT.Exp,
                        scale=rng[:, j : j + 1], accum_out=den[:],
                    )
                    rden = den_pool.tile([P, 1], dt.float32, tag="rden")
                    nc.vector.reciprocal(rden[:], den[:])
                    if t == bt0:
                        pm = pm_pool.tile([P, XB * K], dt.float32, tag="pmac")
                    if MUL_MODE == "act":
                        nc.scalar.activation(
                            pm[:, K * (t % XB) : K * (t % XB + 1)],
                            e[:], AFT.Copy, scale=rden[:],
                        )
                    else:
                        nc.vector.tensor_scalar_mul(
                            pm[:, K * (t % XB) : K * (t % XB + 1)],
                            e[:], rden[:],
                        )
                dst = o_d.ap()[
                    P * XB * tm : P * XB * (tm + 1), :
                ].rearrange("(s p) k -> p s k", s=XB)
                nc.sync.dma_start(
                    dst, pm[:].rearrange("p (s k) -> p s k", s=XB)
                )

    nc.compile()
    return nc


_cache = {}


def _get_nc(npc=NPC):
    if npc not in _cache:
        _cache[npc] = build_bass(npc)
    return _cache[npc]


def kernel(batch: np.ndarray, centroids: np.ndarray) -> np.ndarray:
    from concourse.bass_utils import run_bass_kernel_spmd

    assert batch.shape == (N, D) and centroids.shape == (K, D)
    batch = np.ascontiguousarray(batch, dtype=np.float32)
    centroids = np.ascontiguousarray(centroids, dtype=np.float32)

    nc = _get_nc()
    in_maps = [
        {"x": batch[i * NPC : (i + 1) * NPC], "c": centroids}
        for i in range(NCORES)
    ]
    res = run_bass_kernel_spmd(nc, in_maps, core_ids=list(range(NCORES)))
    return np.concatenate([res.results[i]["o"] for i in range(NCORES)], axis=0)



# revision 7
# speedup vs baseline: 1.1572x; 1.1572x over previous
"""VQ codebook cosine-similarity softmax kernel for Trainium2 (8 NeuronCores).

Computes softmax(cos_sim(batch, centroids)) for batch [131072, 1024] f32 and
centroids [256, 1024] f32, data-parallel over the batch dim across 8 cores.

Per-core pipeline (16384 rows; partition p owns rows [128p, 128p+128), so
every DMA descriptor is a large contiguous DRAM chunk):
  - SWDGE cast-DMA loads x HBM f32 -> SBUF fp16 in 2MB sub-loads with 16KB
    contiguous descriptors; loads are issued first and prefetch XM_BUFS
    groups ahead so the HBM read streams at line rate from t~=1us
  - row norms: one scalar_tensor_tensor(x*x, accum) per 128-row tile,
    round-robined across GpSimd / ACT(Square) / DVE to balance engine load
  - rsqrt of the batched norms via the 0x5f3759df bit trick + Newton (DVE)
  - PE transposes each [128,128] fp16 block; DVE copies PSUM->SBUF; the
    whole loop is software-pipelined by stage (norms one group ahead;
    transpose at tile t, matmul at t-1, exp at t-2) so no engine queue
    head-of-line-blocks on another engine's latest result
  - PE matmul: weights = xT block [128d,128n], moving = cnT [128d,256k],
    accumulating 8 d-chunks into PSUM f32 [128n,256k]
  - softmax: logits = cos in [-1,1] so no max-subtraction needed; ACT
    Exp(scale=1/||x||) emits fp16 e + f32 denominator via accum; batched
    DVE reciprocal; DVE tensor_scalar_mul -> fp16 output tile
  - fp16 stores (1MB, 8KB descriptors); host upcasts to f32
"""

import os
import sys

if "/opt/trn_rl_repo" not in sys.path:
    sys.path.insert(0, "/opt/trn_rl_repo")

import numpy as np

N, D, K = 131072, 1024, 256
NCORES = 8
NPC = N // NCORES  # rows per core
P = 128  # partitions

# norm-engine schedule per position within an XB group, alternating by
# group (comma-separated): v=DVE sts, a=ACT Square. 'v' last so the group
# rsqrt on DVE never waits long on a cross-engine norm. 9a/7v per 16 tiles
# balances ACT (exp+squares) against DVE (copyback+muls+sts).
NORM_SCHED = os.environ.get("KM_NORM_SCHED", "aavaavav,avavavav")
I32_COPY = os.environ.get("KM_I32_COPY", "1") == "1"
XB = int(os.environ.get("KM_XB", "8"))  # tiles per xm buffer / norm group
LSPLIT = int(os.environ.get("KM_LSPLIT", "2"))  # load DMAs per xm buffer
XM_BUFS = int(os.environ.get("KM_XM_BUFS", "4"))
SG = int(os.environ.get("KM_SG", "16"))  # tiles per store DMA
XT_BUFS = int(os.environ.get("KM_XT_BUFS", "4"))
E_BUFS = int(os.environ.get("KM_E_BUFS", "10"))
TPS_BUFS = int(os.environ.get("KM_TPS_BUFS", "2"))
SPS_BUFS = int(os.environ.get("KM_SPS_BUFS", "4"))
OUT_F32 = os.environ.get("KM_OUT_F32", "0") == "1"

RSQRT_MAGIC = 0x5F3759DF


def build_bass(npc=NPC):
    """Build the single-core SPMD program; every core runs this with its own
    x shard. Returns the compiled Bacc object."""
    from contextlib import ExitStack

    import concourse.bacc as bacc
    import concourse.mybir as mybir
    import concourse.tile as tile
    from concourse.masks import make_identity

    dt = mybir.dt
    AFT = mybir.ActivationFunctionType
    Alu = mybir.AluOpType

    nt = npc // P  # row tiles (128)
    assert npc % (P * XB) == 0 and nt % SG == 0
    nxm = nt // XB  # xm groups
    odt = dt.float32 if OUT_F32 else dt.float16

    nc = bacc.Bacc(
        "TRN2", target_bir_lowering=False, debug=False, num_devices=NCORES
    )
    x_d = nc.dram_tensor("x", [npc, D], dt.float32, kind="ExternalInput")
    c_d = nc.dram_tensor("c", [K, D], dt.float32, kind="ExternalInput")
    o_d = nc.dram_tensor("o", [npc, K], odt, kind="ExternalOutput")

    ND = D // P  # d-chunks (8)

    # partition-owns-consecutive-rows views: element (p, s, d) = x[128p+s, d]
    x_v = x_d.ap().rearrange("(p s) d -> p s d", p=P)
    o_v = o_d.ap().rearrange("(p s) k -> p s k", p=P)

    def emit_rsqrt(nc, dst, src, scratch_a, scratch_b):
        """dst = 1/sqrt(src) — bit trick + 3 Newton steps, all on DVE."""
        srci = src.bitcast(dt.int32)
        dsti = dst.bitcast(dt.int32)
        nc.vector.tensor_scalar(dsti, srci, 1, None, Alu.logical_shift_right)
        # magic - x == (x ^ 0xffffffff) + (magic + 1)  (avoids int negate)
        nc.vector.tensor_scalar(dsti, dsti, -1, None, Alu.bitwise_xor)
        nc.vector.tensor_scalar(dsti, dsti, RSQRT_MAGIC + 1, None, Alu.add)
        for _ in range(3):
            nc.vector.tensor_tensor(scratch_a, dst, dst, Alu.mult)
            nc.vector.tensor_tensor(scratch_b, scratch_a, src, Alu.mult)
            nc.vector.tensor_scalar(
                scratch_b, scratch_b, -0.5, 1.5, Alu.mult, Alu.add
            )
            nc.vector.tensor_tensor(dst, dst, scratch_b, Alu.mult)

    with tile.TileContext(nc) as tc, ExitStack() as ctx:
        const = ctx.enter_context(tc.tile_pool(name="const", bufs=1))
        ident = const.tile([P, P], dt.float16)

        # cnT: [128 (d within chunk), ND * K] fp16; chunk b at cols [K*b, K*b+K)
        cnT = const.tile([P, ND * K], dt.float16)
        # per-tile squared row norms, rsqrt'd norms, softmax denominators
        n2 = const.tile([P, nt], dt.float32)
        rng = const.tile([P, nt], dt.float32)
        denscols = const.tile([P, nt], dt.float32)
        rdenscols = const.tile([P, nt], dt.float32)

        xm_pool = ctx.enter_context(tc.tile_pool(name="xm", bufs=XM_BUFS))
        xt_pool = ctx.enter_context(tc.tile_pool(name="xt", bufs=XT_BUFS))
        sq_pool = ctx.enter_context(tc.tile_pool(name="sq", bufs=2))
        e_pool = ctx.enter_context(tc.tile_pool(name="e", bufs=E_BUFS))
        pm_pool = ctx.enter_context(tc.tile_pool(name="pm", bufs=2))
        nrm_pool = ctx.enter_context(tc.tile_pool(name="nrm", bufs=2))
        cprep_pool = ctx.enter_context(tc.tile_pool(name="cprep", bufs=2))
        tps_pool = ctx.enter_context(
            tc.tile_pool(name="tps", bufs=TPS_BUFS, space="PSUM")
        )
        sps_pool = ctx.enter_context(
            tc.tile_pool(name="sps", bufs=SPS_BUFS, space="PSUM")
        )
        cpsum_pool = ctx.enter_context(
            tc.tile_pool(name="cps", bufs=2, space="PSUM")
        )

        xmacs = {}

        def emit_load(m):
            """Cast-load xm group m (XB tiles) in LSPLIT sub-DMAs."""
            xm = xm_pool.tile([P, XB * D], dt.float16, tag="xm")
            sub = XB // LSPLIT
            for j in range(LSPLIT):
                s0 = m * XB + j * sub
                nc.gpsimd.dma_start(
                    xm[:, j * sub * D : (j + 1) * sub * D].rearrange(
                        "p (s d) -> p s d", s=sub
                    ),
                    x_v[:, s0 : s0 + sub, :],
                )
            xmacs[m] = xm

        # prefetch: fill all xm buffers before anything else
        for m in range(min(XM_BUFS, nxm)):
            emit_load(m)

        make_identity(nc, ident[:])

        # ---- centroid prep (~1MB one-time; overlaps the x prefetch) ----
        cn16s = []
        for h in range(K // P):  # 2 halves of the K=256 centroids
            c32 = cprep_pool.tile([P, D], dt.float32, tag="c32")
            nc.sync.dma_start(c32[:], c_d.ap()[P * h : P * (h + 1), :])
            csq = cprep_pool.tile([P, D], dt.float32, tag="csq")
            cn2 = cprep_pool.tile([P, 1], dt.float32, tag="cn2")
            nc.vector.scalar_tensor_tensor(
                csq[:], c32[:], 1.0, c32[:], Alu.mult, Alu.mult,
                accum_out=cn2[:],
            )
            crn = cprep_pool.tile([P, 1], dt.float32, tag="crn")
            csa = cprep_pool.tile([P, 1], dt.float32, tag="csa")
            csb = cprep_pool.tile([P, 1], dt.float32, tag="csb")
            emit_rsqrt(nc, crn[:], cn2[:], csa[:], csb[:])
            cn16 = cprep_pool.tile([P, D], dt.float16, tag="cn16")
            nc.vector.tensor_scalar_mul(cn16[:], c32[:], crn[:])
            cn16s.append(cn16)
        for b in range(ND):  # d-chunk-major so early matmul chunks unblock
            for h in range(K // P):
                pt = cpsum_pool.tile([P, P], dt.float16, tag="ct_ps")
                nc.tensor.transpose(
                    pt[:], cn16s[h][:, P * b : P * (b + 1)], ident[:]
                )
                nc.vector.tensor_copy(
                    cnT[:, K * b + P * h : K * b + P * h + P], pt[:]
                )

        # ---- software-pipelined main loop ----
        tpss, xts, spss, es, pms = {}, {}, {}, {}, {}

        sched_parts = NORM_SCHED.split(",")

        def norm(t):
            xm = xmacs[t // XB]
            xs = xm[:, D * (t % XB) : D * (t % XB + 1)]
            part = sched_parts[(t // XB) % len(sched_parts)]
            eng = part[t % len(part)]
            if eng == "a":
                sq = sq_pool.tile([P, D], dt.float16, tag="sq_a")
                nc.scalar.activation(
                    sq[:], xs, AFT.Square, accum_out=n2[:, t : t + 1]
                )
            else:
                sq = sq_pool.tile([P, D], dt.float16, tag="sq_v")
                nc.vector.scalar_tensor_tensor(
                    sq[:], xs, 1.0, xs, Alu.mult, Alu.mult,
                    accum_out=n2[:, t : t + 1],
                )

        def group_rsqrt(g):
            t0 = g * XB
            nsa = nrm_pool.tile([P, XB], dt.float32, tag="nsa")
            nsb = nrm_pool.tile([P, XB], dt.float32, tag="nsb")
            emit_rsqrt(
                nc, rng[:, t0 : t0 + XB], n2[:, t0 : t0 + XB], nsa[:], nsb[:]
            )

        def transpose(t):
            xm = xmacs[t // XB]
            xs = xm[:, D * (t % XB) : D * (t % XB + 1)]
            tps = tps_pool.tile([P, D], dt.float16, tag="tps")
            for b in range(ND):
                nc.tensor.transpose(
                    tps[:, P * b : P * (b + 1)],
                    xs[:, P * b : P * (b + 1)],
                    ident[:],
                )
            tpss[t] = tps

        def copyback(t):
            xt = xt_pool.tile([P, D], dt.float16, tag="xt")
            tps = tpss.pop(t)
            if I32_COPY:  # bit-move 2 fp16 per elem: halves DVE elem count
                nc.vector.tensor_copy(
                    xt[:].bitcast(dt.int32), tps[:].bitcast(dt.int32)
                )
            else:
                nc.vector.tensor_copy(xt[:], tps[:])
            xts[t] = xt

        def matmul(t):
            xt = xts.pop(t)
            sps = sps_pool.tile([P, K], dt.float32, tag="sps")
            for b in range(ND):
                nc.tensor.matmul(
                    sps[:],
                    xt[:, P * b : P * (b + 1)],
                    cnT[:, K * b : K * (b + 1)],
                    start=(b == 0),
                    stop=(b == ND - 1),
                )
            spss[t] = sps

        def exp(t):
            e = e_pool.tile([P, K], dt.float16, tag="e")
            nc.scalar.activation(
                e[:], spss.pop(t)[:], AFT.Exp,
                scale=rng[:, t : t + 1],
                accum_out=denscols[:, t : t + 1],
            )
            es[t] = e

        def group_recip(g):
            t0 = g * XB
            nc.vector.reciprocal(
                rdenscols[:, t0 : t0 + XB], denscols[:, t0 : t0 + XB]
            )

        def mul(t):
            sg = t // SG
            if t % SG == 0:
                pms[sg] = pm_pool.tile([P, SG * K], odt, tag="pm", name="pm")
            nc.vector.tensor_scalar_mul(
                pms[sg][:, K * (t % SG) : K * (t % SG + 1)],
                es.pop(t)[:],
                rdenscols[:, t : t + 1],
            )

        def store(sg):
            nc.sync.dma_start(
                o_v[:, sg * SG : (sg + 1) * SG, :],
                pms.pop(sg)[:].rearrange("p (s k) -> p s k", s=SG),
            )

        # stage offsets: norm at s (one group ahead), transpose at s-XB,
        # matmul at s-XB-1, exp at s-XB-2; rsqrt for group g at step
        # (g+1)*XB (after the group's norms, before any exp needs it)
        for s in range(nt + XB + 2):
            tt = s - XB
            if 0 <= tt < nt:
                transpose(tt)
                copyback(tt)
            tm = s - XB - 1
            if 0 <= tm < nt:
                matmul(tm)
            te = s - XB - 2
            if 0 <= te < nt:
                exp(te)
                if te % XB == XB - 1:
                    group_recip(te // XB)
                    for tq in range(te - XB + 1, te + 1):
                        mul(tq)
                    if (te + 1) % SG == 0:
                        store(te // SG)
            if s < nt:
                norm(s)
                if s % XB == XB - 1:
                    m = s // XB + XM_BUFS
                    if m < nxm:
                        emit_load(m)
            if s % XB == 0 and 0 < s <= nt:
                group_rsqrt(s // XB - 1)

    nc.compile()
    return nc


_cache = {}


def _get_nc(npc=NPC):
    if npc not in _cache:
        _cache[npc] = build_bass(npc)
    return _cache[npc]


def kernel(batch: np.ndarray, centroids: np.ndarray) -> np.ndarray:
    from concourse.bass_utils import run_bass_kernel_spmd

    assert batch.shape == (N, D) and centroids.shape == (K, D)
    batch = np.ascontiguousarray(batch, dtype=np.float32)
    centroids = np.ascontiguousarray(centroids, dtype=np.float32)

    nc = _get_nc()
    in_maps = [
        {"x": batch[i * NPC : (i + 1) * NPC], "c": centroids}
        for i in range(NCORES)
    ]
    res = run_bass_kernel_spmd(nc, in_maps, core_ids=list(range(NCORES)))
    out = np.concatenate(
        [np.asarray(res.results[i]["o"]) for i in range(NCORES)], axis=0
    )
    return out.astype(np.float32, copy=False)
